# revision 23
# baseline (speedup 1.0000x reference)
"""Trainium-2 Bass kernel for nn_EnhancedGNNEncoder (4-layer bipartite GNN).

8 NeuronCores, one SPMD Bass program. Nodes canonically sharded; edges
sharded by destination owner with a per-core-uniform chunk schedule
(64-node dst windows x 25k-row src banks). Message rows fetched with
dma_gather (4 SWDGE queues); segment-sum via host-built one-hot matrices
on the tensor engine accumulating [64,65] PSUM windows (degree in col 64);
per-pass message tables exchanged with AllGather collectives.
"""
import numpy as np
import ml_dtypes

import concourse.bacc as bacc
import concourse.mybir as mybir
from concourse.tile import TileContext
from concourse.bass_utils import run_bass_kernel_spmd
from concourse._compat import get_trn_type, cdiv
from concourse.library_config import mlp as mlp_lib
from concourse.masks import make_identity

BF16 = ml_dtypes.bfloat16
NCORES = 8
WIN = 64
BANK = 25000
GCALL = 8
D = 64
LN_EPS = 1e-3
LAYERS = 4
NV, NC_, NK = 100000, 50000, 20000
EF = 8
AF = mybir.ActivationFunctionType
ALU = mybir.AluOpType
AX = mybir.AxisListType

DIRS = ["vc0", "vc1", "vk0", "vk1"]
SRCT = {"vc0": "var", "vc1": "cons", "vk0": "var", "vk1": "cut"}
DSTT = {"vc0": "cons", "vc1": "var", "vk0": "cut", "vk1": "var"}
NTYPE = {"var": NV, "cons": NC_, "cut": NK}
FD = {"var": 19, "cons": 5, "cut": 30}
TSRC = {"vc0": "tvar_vc", "vc1": "tcons_vc", "vk0": "tvar_vk", "vk1": "tcut_vk"}
TDST = {"vc0": "tcons_vc", "vc1": "tvar_vc", "vk0": "tcut_vk", "vk1": "tvar_vk"}


def _wrap_idx(flat):
    n = len(flat)
    a = np.zeros((16, cdiv(n, 16)), dtype=np.int16)
    a[np.arange(n) % 16, np.arange(n) // 16] = flat
    return np.tile(a, (8, 1))


class DirSchedule:
    """Uniform schedule for one direction. Chunk streams are stored in
    *bank-stream* column order: col(k) = bank_base[bank(k)] + pos_in_bank(k)."""

    def __init__(self, src, dst, ef, n_src, n_dst, name):
        self.name, self.n_src, self.n_dst = name, n_src, n_dst
        self.shard = n_dst // NCORES
        self.nwin = cdiv(self.shard, WIN)
        self.nbank = cdiv(n_src, BANK)
        owner = dst // self.shard
        dloc = dst - owner * self.shard
        w_of = dloc // WIN
        b_of = src // BANK

        per = {}
        cpb = np.ones(self.nbank, dtype=np.int64)
        for c in range(NCORES):
            m = np.flatnonzero(owner == c)
            key = w_of[m].astype(np.int64) * self.nbank + b_of[m]
            e = m[np.argsort(key, kind="stable")]
            per[c] = e
            for bk in range(self.nbank):
                sel = e[b_of[e] == bk]
                if sel.size:
                    cnt = np.bincount(w_of[sel], minlength=self.nwin)
                    cpb[bk] = max(cpb[bk], cdiv(int(cnt.max()), 128))
        self.cpb = [int(x) for x in cpb]
        self.cpw = sum(self.cpb)
        self.nchunk = self.nwin * self.cpw
        slot_bank = []
        for bk in range(self.nbank):
            slot_bank += [bk] * self.cpb[bk]
        self.slot_bank = slot_bank
        # bank-stream maps
        self.bank_nchunk = [self.nwin * self.cpb[bk] for bk in range(self.nbank)]
        self.bank_base = np.concatenate([[0], np.cumsum(self.bank_nchunk)])[:-1]
        self.colmap = np.zeros(self.nchunk, dtype=np.int64)
        pos = [0] * self.nbank
        for k in range(self.nchunk):
            bk = slot_bank[k % self.cpw]
            self.colmap[k] = self.bank_base[bk] + pos[bk]
            pos[bk] += 1

        self.efp = np.zeros((NCORES, EF + 1, self.nchunk * 128), dtype=np.float32)
        self.efp[:, EF, :] = 1.0
        self.src_idx, self.dst_idx, self.wrel_idx = [], [], []
        for c in range(NCORES):
            e = per[c]
            sflat = np.zeros(self.nchunk * 128, dtype=np.int16)
            dflat = np.zeros(self.nchunk * 128, dtype=np.int16)
            # window-relative dloc; padded slots get -1 (matches no iota row)
            wflat = np.full(self.nchunk * 128, -1, dtype=np.int16)
            for wv in range(self.nwin):
                base = wv * self.cpw
                off = 0
                we = e[w_of[e] == wv]
                for bk in range(self.nbank):
                    sel = we[b_of[we] == bk]
                    for j in range(self.cpb[bk]):
                        k = base + off + j
                        col = self.colmap[k]
                        part = sel[j * 128:(j + 1) * 128]
                        n = len(part)
                        if n:
                            self.efp[c, :EF, col * 128:col * 128 + n] = ef[part].T
                            sflat[col * 128:col * 128 + n] = (
                                src[part] - bk * BANK).astype(np.int16)
                            dflat[col * 128:col * 128 + n] = dloc[part].astype(np.int16)
                            wflat[col * 128:col * 128 + n] = (
                                dloc[part] - wv * WIN).astype(np.int16)
                    off += self.cpb[bk]
            self.src_idx.append(_wrap_idx(sflat))
            self.dst_idx.append(_wrap_idx(dflat))
            self.wrel_idx.append(wflat.reshape(self.nchunk, 128).T.copy())
        self.idx_cols = cdiv(self.nchunk * 128, 16)
        self.npiece = cdiv(self.nchunk, GCALL)  # pieces over the bank stream
        # piece -> bank (pieces never straddle banks if bank_nchunk % GCALL == 0;
        # pad bank streams to GCALL multiples would change nchunk; instead allow
        # a piece to straddle banks only if impossible -- enforce alignment:
        assert all(n % 1 == 0 for n in self.bank_nchunk)
        # straddling pieces split at emission time into per-bank calls.


def build_schedules(inputs):
    vc, vk = inputs["var_cons_edges"], inputs["var_cut_edges"]
    efvc, efvk = inputs["var_cons_edge_features"], inputs["var_cut_edge_features"]
    return {
        "vc0": DirSchedule(vc[0], vc[1], efvc, NV, NC_, "vc0"),
        "vc1": DirSchedule(vc[1], vc[0], efvc, NC_, NV, "vc1"),
        "vk0": DirSchedule(vk[0], vk[1], efvk, NV, NK, "vk0"),
        "vk1": DirSchedule(vk[1], vk[0], efvk, NK, NV, "vk1"),
    }


def build_kernel(scheds):
    f32, bf16, i16 = mybir.dt.float32, mybir.dt.bfloat16, mybir.dt.int16
    nc = bacc.Bacc(get_trn_type() or "TRN2", num_swdge_queues=4)
    RG = [list(range(NCORES))]
    ins = {}

    def I(name, shape, dt=f32):
        ins[name] = nc.dram_tensor(name, shape, dt, kind="ExternalInput")
        return ins[name]

    for t in ("var", "cons", "cut"):
        I(f"featT_{t}", [FD[t] + 1, NTYPE[t] // NCORES])
        I(f"emb_{t}_w1", [FD[t] + 1, D])
        I(f"emb_{t}_w2", [D, D])
        I(f"emb_{t}_b2", [1, D])
    for d in DIRS:
        s = scheds[d]
        I(f"sidx_{d}", [128, s.idx_cols], i16)
        I(f"didx_{d}", [128, s.idx_cols], i16)
        I(f"widx_{d}", [128, s.nchunk], bf16)
        I(f"efT_{d}", [EF + 1, s.nchunk * 128])
    I("iota_row", [128, WIN], bf16)
    for et in ("vc", "vk"):
        I(f"ew_{et}_w1v", [D, D])
        I(f"ew_{et}_w1o", [D, D])
        I(f"ew_{et}_w1e", [EF + 1, D])
        I(f"ew_{et}_w2t", [1, GCALL * D])
        I(f"ew_{et}_b2", [1, 1])
    I("mp_msg_w1", [16, D, D]); I("mp_msg_b1", [16, 1, D])
    I("mp_msg_w2", [16, D, D]); I("mp_msg_b2", [16, 1, D])
    I("mp_gate_w", [16, 2 * D, D], bf16); I("mp_gate_b", [16, 1, D])
    I("mp_upd_w1", [16, 2 * D, D], bf16); I("mp_upd_b1", [16, 1, D])
    I("mp_upd_w2", [16, D, D], bf16); I("mp_upd_b2", [16, 1, D])
    I("mp_ln_g", [16, 1, D]); I("mp_ln_b", [16, 1, D])

    out_t = nc.dram_tensor("out_cut", [NK // NCORES, D], f32, kind="ExternalOutput")

    # hT node tables live in SBUF for the whole kernel (col offsets per type)
    HOFF = {"var": 0, "cons": NV // NCORES, "cut": NV // NCORES + NC_ // NCORES}
    HTOT = (NV + NC_ + NK) // NCORES
    tcc_in, tcc_out = {}, {}
    for nm, t in (("tvar_vc", "var"), ("tcons_vc", "cons"),
                  ("tvar_vk", "var"), ("tcut_vk", "cut")):
        tcc_in[nm] = nc.dram_tensor(f"cci_{nm}", [NTYPE[t] // NCORES, D], f32)
        tcc_out[nm] = nc.dram_tensor(f"cco_{nm}", [NTYPE[t], D], f32,
                                     addr_space="Shared")
    msg_in, msg_out = {}, {}
    for l in range(4):
        for d in DIRS:
            msg_in[(l, d)] = nc.dram_tensor(
                f"mci{l}{d}", [NTYPE[SRCT[d]] // NCORES, D], f32)
            msg_out[(l, d)] = nc.dram_tensor(
                f"mco{l}{d}", [NTYPE[SRCT[d]], D], f32, addr_space="Shared")

    with TileContext(nc) as tc:
        with (
            tc.tile_pool(name="const", bufs=1) as cp,
            tc.tile_pool(name="sb", bufs=2) as sb,
            tc.tile_pool(name="gp", bufs=2) as gp,
            tc.tile_pool(name="mgp", bufs=6) as mgp,
            tc.tile_pool(name="wp", bufs=6) as wp,
            tc.tile_pool(name="ps", bufs=2, space="PSUM") as ps,
            tc.tile_pool(name="psw", bufs=3, space="PSUM") as psw,
            tc.tile_pool(name="pst", bufs=2, space="PSUM") as pst,
        ):
            nc.gpsimd.load_library(mlp_lib)
            ident = cp.tile([128, 128], f32)
            make_identity(nc, ident)
            ones1 = cp.tile([1, 128], f32)
            nc.gpsimd.memset(ones1[:], 1.0)
            eps64 = cp.tile([128, 1], f32)
            nc.gpsimd.memset(eps64[:], float(D) * LN_EPS)
            inv64 = cp.tile([128, 1], f32)
            nc.gpsimd.memset(inv64[:], 1.0 / D)
            hT_sb = cp.tile([D, HTOT], f32)

            def hsl(t, a, b):
                return hT_sb[:, HOFF[t] + a:HOFF[t] + b]

            # iota row 0..WIN-1 on all partitions + per-direction
            # window-relative dst indices (-1..63); small ints, exact in bf16
            iotaf = cp.tile([128, WIN], bf16)
            nc.sync.dma_start(iotaf[:], ins["iota_row"][:])
            didxw = {}
            for d in DIRS:
                s = scheds[d]
                dwt = cp.tile([128, s.nchunk], bf16, tag=f"dw{d}")
                nc.sync.dma_start(dwt[:], ins[f"widx_{d}"][:])
                didxw[d] = dwt

            def mmul(o, lt, r, st=True, sp=True):
                nc.tensor.matmul(o, lt, r, start=st, stop=sp)

            def col_of(row_dram, n, tag="colc"):
                rr = sb.tile([1, 128], f32, tag="colr")
                nc.sync.dma_start(rr[:, :n], row_dram)
                p = pst.tile([128, 128], f32, tag="tr")
                nc.tensor.transpose(p[:n, :1], rr[:1, :n], ident[:1, :1])
                c = sb.tile([128, 1], f32, tag=tag)
                nc.scalar.copy(c[:n, :], p[:n, :1])
                return c

            def bcast128(row_dram, n, tagsuf):
                rr = sb.tile([1, 128], f32, tag="bcr")
                nc.sync.dma_start(rr[:, :n], row_dram)
                p = pst.tile([128, 128], f32, tag="tr")
                mmul(p[:, :n], ones1[:], rr[:, :n])
                t = cp.tile([128, D], f32, tag="bc" + tagsuf)
                nc.vector.tensor_copy(t[:, :n], p[:, :n])
                return t

            # ---------------- P1: embeddings ----------------
            for t in ("var", "cons", "cut"):
                w1 = sb.tile([FD[t] + 1, D], f32, tag="ew1")
                nc.sync.dma_start(w1[:], ins[f"emb_{t}_w1"][:])
                w2 = sb.tile([D, D], f32, tag="ew2")
                nc.sync.dma_start(w2[:], ins[f"emb_{t}_w2"][:])
                b2c = col_of(ins[f"emb_{t}_b2"][:, :], D)
                sh = NTYPE[t] // NCORES
                for g0 in range(0, sh, 512):
                    gw = min(512, sh - g0)
                    ft = sb.tile([FD[t] + 1, 512], f32, tag="ft")
                    nc.sync.dma_start(ft[:, :gw], ins[f"featT_{t}"][:, g0:g0 + gw])
                    p1 = ps.tile([128, 512], f32, tag="mm")
                    mmul(p1[:D, :gw], w1[:], ft[:, :gw])
                    r1 = sb.tile([D, 512], f32, tag="r1")
                    nc.scalar.activation(r1[:, :gw], p1[:D, :gw], AF.Relu)
                    p2 = ps.tile([128, 512], f32, tag="mm")
                    mmul(p2[:D, :gw], w2[:], r1[:, :gw])
                    nc.vector.tensor_scalar_add(hsl(t, g0, g0 + gw),
                                                p2[:D, :gw], b2c[:D, :])

            # ---------------- P2a: t tables + AllGather ----------------
            for nm, t, wn in (("tvar_vc", "var", "ew_vc_w1v"),
                              ("tcons_vc", "cons", "ew_vc_w1o"),
                              ("tvar_vk", "var", "ew_vk_w1v"),
                              ("tcut_vk", "cut", "ew_vk_w1o")):
                wt = sb.tile([D, D], f32, tag="ew2")
                nc.sync.dma_start(wt[:], ins[wn][:])
                sh = NTYPE[t] // NCORES
                for g0 in range(0, sh, 512):
                    gw = min(512, sh - g0)
                    pt = ps.tile([128, 512], f32, tag="mm")
                    mmul(pt[:D, :gw], wt[:], hsl(t, g0, g0 + gw))
                    tt = sb.tile([D, 512], f32, tag="r1")
                    nc.vector.tensor_copy(tt[:, :gw], pt[:D, :gw])
                    for j0 in range(0, gw, 128):
                        jw = min(128, gw - j0)
                        pn = pst.tile([128, 128], f32, tag="tr")
                        nc.tensor.transpose(pn[:jw, :D], tt[:, j0:j0 + jw], ident[:D, :D])
                        nb = sb.tile([128, D], f32, tag="nb")
                        nc.scalar.copy(nb[:jw, :], pn[:jw, :D])
                        nc.sync.dma_start(tcc_in[nm][g0 + j0:g0 + j0 + jw, :],
                                          nb[:jw, :])
                nc.gpsimd.collective_compute(
                    "AllGather", ALU.bypass, ins=[tcc_in[nm][:]],
                    outs=[tcc_out[nm][:]], replica_groups=RG)

            # ---------------- P2b: edge weights ----------------

            ew_tiles, r_tiles = {}, {}
            for d in DIRS:
                s = scheds[d]
                et = "vc" if d[:2] == "vc" else "vk"
                w1e = sb.tile([EF + 1, D], f32, tag="w1e")
                nc.sync.dma_start(w1e[:], ins[f"ew_{et}_w1e"][:])
                w2b = cp.tile([128, GCALL, D], f32, tag=f"w2b{et}")
                w2r = sb.tile([1, GCALL * D], f32, tag="w2r")
                nc.sync.dma_start(w2r[:], ins[f"ew_{et}_w2t"][:])
                for hh in range(1):
                    pw = ps.tile([128, 512], f32, tag="mm")
                    mmul(pw[:], ones1[:], w2r[:])
                    nc.vector.tensor_copy(
                        w2b[:].rearrange("p a b -> p (a b)")[:], pw[:])
                b2b = cp.tile([128, 1], f32, tag=f"eb2{et}")
                pb2 = ps.tile([128, 512], f32, tag="mm")
                b2r = sb.tile([1, 1], f32, tag="b2r1")
                nc.sync.dma_start(b2r[:], ins[f"ew_{et}_b2"][:])
                mmul(pb2[:, :1], ones1[:], b2r[:])
                nc.vector.tensor_copy(b2b[:], pb2[:, :1])

                ew_sb = cp.tile([128, s.nchunk], bf16, tag=f"ews{d}")
                ew_tiles[d] = ew_sb
                for p0 in range(0, s.nchunk, GCALL):
                    pn = min(GCALL, s.nchunk - p0)
                    # source-side gather: split by bank within the piece
                    sit = sb.tile([128, GCALL * 8], i16, tag="sit")
                    nc.sync.dma_start(sit[:, :pn * 8],
                                      ins[f"sidx_{d}"][:, p0 * 8:(p0 + pn) * 8])
                    dit = sb.tile([128, GCALL * 8], i16, tag="dit")
                    nc.sync.dma_start(dit[:, :pn * 8],
                                      ins[f"didx_{d}"][:, p0 * 8:(p0 + pn) * 8])
                    g1 = gp.tile([128, GCALL, D], f32, tag="sg")
                    c0 = p0
                    while c0 < p0 + pn:
                        bk = int(np.searchsorted(s.bank_base, c0, side="right") - 1)
                        bend = (s.bank_base[bk + 1] if bk + 1 < s.nbank
                                else s.nchunk)
                        cn = min(p0 + pn, bend) - c0
                        nrow = min(BANK, s.n_src - bk * BANK)
                        nc.gpsimd.dma_gather(
                            g1[:, c0 - p0:c0 - p0 + cn, :],
                            tcc_out[TSRC[d]][bk * BANK:bk * BANK + nrow, :],
                            sit[:, (c0 - p0) * 8:(c0 - p0 + cn) * 8],
                            cn * 128, cn * 128, D, single_packet=True,
                            queue_num=(c0 // GCALL) % 2)
                        c0 += cn
                    g2 = gp.tile([128, GCALL, D], f32, tag="dg")
                    nc.gpsimd.dma_gather(
                        g2[:, :pn, :], tcc_in[TDST[d]][:, :],
                        dit[:, :pn * 8],
                        pn * 128, pn * 128, D, single_packet=True,
                        queue_num=2 + (p0 // GCALL) % 2)
                    eft = sb.tile([EF + 1, GCALL * 128], f32, tag="eft")
                    nc.sync.dma_start(eft[:, :pn * 128],
                                      ins[f"efT_{d}"][:, p0 * 128:(p0 + pn) * 128])
                    pre = sb.tile([128, GCALL, D], f32, tag="pre")
                    for hh in range(0, pn, 8):
                        hn = min(8, pn - hh)
                        pe = ps.tile([128, 512], f32, tag="mm")
                        pev = pe[:].rearrange("p (a b) -> p a b", b=D)
                        for j in range(hn):
                            col = hh + j
                            mmul(pev[:, j, :],
                                 eft[:, col * 128:(col + 1) * 128],
                                 w1e[:])
                        nc.vector.tensor_tensor(
                            out=pre[:, hh:hh + hn, :], in0=g1[:, hh:hh + hn, :],
                            in1=pev[:, :hn, :], op=ALU.add)
                    nc.vector.tensor_tensor(out=pre[:, :pn, :], in0=pre[:, :pn, :],
                                            in1=g2[:, :pn, :], op=ALU.add)
                    rl = sb.tile([128, GCALL, D], f32, tag="rl")
                    nc.scalar.activation(rl[:, :pn, :], pre[:, :pn, :], AF.Relu)
                    nc.vector.tensor_tensor(out=rl[:, :pn, :], in0=rl[:, :pn, :],
                                            in1=w2b[:, :pn, :], op=ALU.mult)
                    sm = sb.tile([128, GCALL], f32, tag="sm")
                    nc.vector.reduce_sum(sm[:, :pn], rl[:, :pn, :], axis=AX.X)
                    nc.scalar.activation(ew_sb[:, p0:p0 + pn], sm[:, :pn],
                                         AF.Sigmoid, bias=b2b[:])
                rt_ = cp.tile([WIN, s.nwin], f32, tag=f"r{d}")
                r_tiles[d] = rt_

            # ---------------- P3: layers ----------------
            for l in range(LAYERS):
                for di, d in enumerate(DIRS):
                    i = l * 4 + di
                    s = scheds[d]
                    st, dt_ = SRCT[d], DSTT[d]
                    ssh, dsh = NTYPE[st] // NCORES, NTYPE[dt_] // NCORES

                    # --- msg mlp on own src shard, to node-major msg_in ---
                    mw1 = sb.tile([D, D], f32, tag="mw1")
                    nc.sync.dma_start(mw1[:], ins["mp_msg_w1"][i])
                    mw2 = sb.tile([D, D], f32, tag="mw2")
                    nc.sync.dma_start(mw2[:], ins["mp_msg_w2"][i])
                    mb1 = col_of(ins["mp_msg_b1"][i], D, "cmb1")
                    mb2 = col_of(ins["mp_msg_b2"][i], D, "cmb2")
                    for g0 in range(0, ssh, 512):
                        gw = min(512, ssh - g0)
                        p1 = ps.tile([128, 512], f32, tag="mm")
                        mmul(p1[:D, :gw], mw1[:], hsl(st, g0, g0 + gw))
                        r1 = sb.tile([D, 512], f32, tag="r1")
                        nc.scalar.activation(r1[:, :gw], p1[:D, :gw], AF.Relu,
                                             bias=mb1[:D, :])
                        p2 = ps.tile([128, 512], f32, tag="mm")
                        mmul(p2[:D, :gw], mw2[:], r1[:, :gw])
                        mt = sb.tile([D, 512], f32, tag="h1")
                        nc.vector.tensor_scalar_add(mt[:, :gw], p2[:D, :gw], mb2[:D, :])
                        for j0 in range(0, gw, 128):
                            jw = min(128, gw - j0)
                            pn_ = pst.tile([128, 128], f32, tag="tr")
                            nc.tensor.transpose(pn_[:jw, :D], mt[:, j0:j0 + jw],
                                                ident[:D, :D])
                            nb = sb.tile([128, D], f32, tag="nb")
                            nc.scalar.copy(nb[:jw, :], pn_[:jw, :D])
                            nc.sync.dma_start(
                                msg_in[(l, d)][g0 + j0:g0 + j0 + jw, :], nb[:jw, :])
                    nc.gpsimd.collective_compute(
                        "AllGather", ALU.bypass, ins=[msg_in[(l, d)][:]],
                        outs=[msg_out[(l, d)][:]], replica_groups=RG)

                    # --- update-phase constants (bf16 weights) ---
                    gwt = sb.tile([2 * D, D], bf16, tag="gwt")
                    nc.sync.dma_start(gwt[:], ins["mp_gate_w"][i])
                    uw1 = sb.tile([2 * D, D], bf16, tag="uw1")
                    nc.sync.dma_start(uw1[:], ins["mp_upd_w1"][i])
                    uw2 = sb.tile([D, D], bf16, tag="uw2")
                    nc.sync.dma_start(uw2[:], ins["mp_upd_w2"][i])
                    gb = col_of(ins["mp_gate_b"][i], D, "cgb")
                    ub1 = col_of(ins["mp_upd_b1"][i], D, "cub1")
                    ub2 = col_of(ins["mp_upd_b2"][i], D, "cub2")
                    lng = bcast128(ins["mp_ln_g"][i], D, "g")
                    lnb = bcast128(ins["mp_ln_b"][i], D, "b")

                    # --- edge phase + fused update every 8 windows ---
                    ew_sb, r_t = ew_tiles[d], r_tiles[d]
                    pieces = {}

                    def get_piece(pidx, d=d, s=s, ew_sb=ew_sb, pieces=pieces, l=l):
                        if pidx in pieces:
                            return pieces[pidx]
                        p0 = pidx * GCALL
                        pn = min(GCALL, s.nchunk - p0)
                        sit = sb.tile([128, GCALL * 8], i16, tag="sit")
                        nc.sync.dma_start(sit[:, :pn * 8],
                                          ins[f"sidx_{d}"][:, p0 * 8:(p0 + pn) * 8])
                        g = mgp.tile([128, GCALL, D], f32, tag="mg")
                        c0 = p0
                        while c0 < p0 + pn:
                            bk = int(np.searchsorted(s.bank_base, c0, "right") - 1)
                            bend = (s.bank_base[bk + 1] if bk + 1 < s.nbank
                                    else s.nchunk)
                            cn = min(p0 + pn, bend) - c0
                            nrow = min(BANK, s.n_src - bk * BANK)
                            nc.gpsimd.dma_gather(
                                g[:, c0 - p0:c0 - p0 + cn, :],
                                msg_out[(l, d)][bk * BANK:bk * BANK + nrow, :],
                                sit[:, (c0 - p0) * 8:(c0 - p0 + cn) * 8],
                                cn * 128, cn * 128, D, single_packet=True,
                                queue_num=pidx % 4)
                            c0 += cn
                        wt = wp.tile([128, GCALL, WIN], bf16, tag="wt")
                        nc.vector.tensor_tensor(
                            out=wt[:, :pn, :],
                            in0=didxw[d][:, p0:p0 + pn, None].to_broadcast(
                                [128, pn, WIN]),
                            in1=iotaf[:, None, :].to_broadcast([128, pn, WIN]),
                            op=ALU.is_equal)
                        wm = wp.tile([128, GCALL, D + 1], bf16, tag="wm")
                        nc.vector.tensor_tensor(
                            out=wm[:, :pn, 0:D], in0=g[:, :pn, :],
                            in1=ew_sb[:, p0:p0 + pn, None].to_broadcast(
                                [128, pn, D]),
                            op=ALU.mult)
                        nc.vector.tensor_copy(wm[:, :pn, D], ew_sb[:, p0:p0 + pn])
                        pieces[pidx] = (wt, wm)
                        if len(pieces) > 5:
                            old = sorted(pieces)[0]
                            if old != pidx:
                                del pieces[old]
                        return pieces[pidx]

                    grp_aggT = None
                    for wv in range(s.nwin):
                        gi = wv % 8
                        if gi == 0:
                            grp_aggT = sb.tile([D, 512], f32, tag="gaggT")
                        pw = psw.tile([WIN, D + 1], f32, tag="win")
                        for j in range(s.cpw):
                            k = wv * s.cpw + j
                            col = int(s.colmap[k])
                            wt, wm = get_piece(col // GCALL)
                            sl = col % GCALL
                            mmul(pw[:], wt[:, sl, :], wm[:, sl, :],
                                 st=(j == 0), sp=(j == s.cpw - 1))
                        if l == 0:
                            dg = sb.tile([WIN, 1], f32, tag="dg1")
                            nc.vector.tensor_scalar_max(dg[:], pw[:, D:D + 1], 1.0)
                            nc.vector.reciprocal(r_t[:, wv:wv + 1], dg[:])
                        agg = sb.tile([WIN, D], f32, tag="aggnm")
                        nc.scalar.activation(agg[:], pw[:, 0:D], AF.Copy,
                                             scale=r_t[:, wv:wv + 1])
                        pt_ = pst.tile([128, 128], f32, tag="tr")
                        nc.tensor.transpose(pt_[:D, :WIN], agg[:], ident[:WIN, :WIN])
                        nc.vector.tensor_copy(grp_aggT[:, gi * WIN:(gi + 1) * WIN],
                                              pt_[:D, :WIN])

                        if gi == 7 or wv == s.nwin - 1:
                            ng = gi + 1
                            n0 = (wv - gi) * WIN
                            nn = ng * WIN
                            hTg = sb.tile([D, 512], f32, tag="hTg")
                            nc.vector.tensor_copy(
                                hTg[:, :min(nn, dsh - n0)],
                                hsl(dt_, n0, min(n0 + nn, dsh)))
                            combT = sb.tile([2 * D, 512], bf16, tag="combT")
                            nc.vector.tensor_copy(combT[0:D, :nn],
                                                  grp_aggT[:, :nn])
                            nc.vector.tensor_copy(combT[D:2 * D, :nn],
                                                  hTg[:, :nn])
                            pg = ps.tile([128, 512], f32, tag="mm")
                            mmul(pg[:D, :nn], gwt[:], combT[:, :nn])
                            gt = sb.tile([D, 512], f32, tag="gt")
                            nc.scalar.activation(gt[:, :nn], pg[:D, :nn],
                                                 AF.Sigmoid, bias=gb[:D, :])
                            pu = ps.tile([128, 512], f32, tag="mm")
                            mmul(pu[:D, :nn], uw1[:], combT[:, :nn])
                            ru = sb.tile([D, 512], bf16, tag="ru")
                            nc.scalar.activation(ru[:, :nn], pu[:D, :nn], AF.Relu,
                                                 bias=ub1[:D, :])
                            pu2 = ps.tile([128, 512], f32, tag="mm")
                            mmul(pu2[:D, :nn], uw2[:], ru[:, :nn])
                            ut = sb.tile([D, 512], f32, tag="ut")
                            nc.vector.tensor_scalar_add(ut[:, :nn], pu2[:D, :nn],
                                                        ub2[:D, :])
                            # out = h + g*(u - h)
                            nc.vector.tensor_tensor(out=ut[:, :nn], in0=ut[:, :nn],
                                                    in1=hTg[:, :nn],
                                                    op=ALU.subtract)
                            nc.vector.tensor_tensor(out=ut[:, :nn], in0=ut[:, :nn],
                                                    in1=gt[:, :nn], op=ALU.mult)
                            nc.vector.tensor_tensor(out=ut[:, :nn], in0=ut[:, :nn],
                                                    in1=hTg[:, :nn],
                                                    op=ALU.add)
                            # (residual uses the f32 hTg staging copy)
                            # LN: per-window node-major blocks on 64 partitions
                            xb = sb.tile([WIN, 8, D], f32, tag="xb")
                            for w2 in range(ng):
                                ptp = pst.tile([128, 128], f32, tag="tr")
                                nc.tensor.transpose(
                                    ptp[0:WIN, :D],
                                    ut[:, w2 * WIN:(w2 + 1) * WIN], ident[:D, :D])
                                nc.vector.tensor_copy(xb[:, w2, :], ptp[:WIN, :D])
                            m_ = sb.tile([WIN, 8], f32, tag="mln")
                            nc.vector.reduce_sum(m_[:, :ng], xb[:, :ng, :],
                                                 axis=AX.X)
                            nc.vector.tensor_scalar_mul(m_[:, :ng],
                                                        m_[:, :ng], inv64[:WIN, :])
                            for w2 in range(ng):
                                nc.vector.tensor_scalar_sub(xb[:, w2, :],
                                                            xb[:, w2, :],
                                                            m_[:, w2:w2 + 1])
                            sq = sb.tile([WIN, 8, D], f32, tag="sq")
                            nc.scalar.activation(sq[:, :ng, :], xb[:, :ng, :],
                                                 AF.Square)
                            v_ = sb.tile([WIN, 8], f32, tag="vln")
                            nc.vector.reduce_sum(v_[:, :ng], sq[:, :ng, :],
                                                 axis=AX.X)
                            nc.vector.tensor_scalar_add(v_[:, :ng],
                                                        v_[:, :ng], eps64[:WIN, :])
                            nc.scalar.activation(v_[:, :ng], v_[:, :ng], AF.Sqrt)
                            nc.vector.reciprocal(v_[:, :ng], v_[:, :ng])
                            for w2 in range(ng):
                                nc.vector.tensor_scalar_mul(xb[:, w2, :],
                                                            xb[:, w2, :],
                                                            v_[:, w2:w2 + 1])
                            nc.vector.tensor_tensor(out=xb[:, :ng, :],
                                                    in0=xb[:, :ng, :],
                                                    in1=lng[:WIN, None, :].to_broadcast(
                                                        [WIN, ng, D]),
                                                    op=ALU.mult)
                            nc.vector.tensor_tensor(out=xb[:, :ng, :],
                                                    in0=xb[:, :ng, :],
                                                    in1=lnb[:WIN, None, :].to_broadcast(
                                                        [WIN, ng, D]),
                                                    op=ALU.add)
                            # transpose back to feature-major, store into hT_sb
                            for w2 in range(ng):
                                c0_ = n0 + w2 * WIN
                                cw_ = min(WIN, dsh - c0_)
                                if cw_ <= 0:
                                    break
                                pth = pst.tile([128, 128], f32, tag="tr")
                                nc.tensor.transpose(pth[:D, :WIN], xb[:, w2, :],
                                                    ident[:WIN, :WIN])
                                nc.vector.tensor_copy(
                                    hsl(dt_, c0_, c0_ + cw_), pth[:D, :cw_])

            # ---------------- output: h_cut node-major ----------------
            csh = NK // NCORES
            for g0 in range(0, csh, 512):
                gw = min(512, csh - g0)
                for j0 in range(0, gw, 128):
                    jw = min(128, gw - j0)
                    po = pst.tile([128, 128], f32, tag="tr")
                    nc.tensor.transpose(po[:jw, :D],
                                        hsl("cut", g0 + j0, g0 + j0 + jw),
                                        ident[:D, :D])
                    nb = sb.tile([128, D], f32, tag="nb")
                    nc.scalar.copy(nb[:jw, :], po[:jw, :D])
                    nc.sync.dma_start(out_t[g0 + j0:g0 + j0 + jw, :], nb[:jw, :])

    nc.compile()
    return nc


def make_inputs(inputs, scheds):
    """Build the per-core input maps from the model inputs + schedules."""
    maps = [dict() for _ in range(NCORES)]
    feats = {"var": "variable_features", "cons": "constraint_features",
             "cut": "cut_features"}
    for t in ("var", "cons", "cut"):
        f = np.asarray(inputs[feats[t]], dtype=np.float32)
        sh = NTYPE[t] // NCORES
        w1 = np.concatenate([np.asarray(inputs[f"{t}_w1"]),
                             np.asarray(inputs[f"{t}_b1"])[None, :]], axis=0)
        for c in range(NCORES):
            ft = np.ones((FD[t] + 1, sh), dtype=np.float32)
            ft[:FD[t], :] = f[c * sh:(c + 1) * sh].T
            maps[c][f"featT_{t}"] = ft
            maps[c][f"emb_{t}_w1"] = np.ascontiguousarray(w1, dtype=np.float32)
            maps[c][f"emb_{t}_w2"] = np.asarray(inputs[f"{t}_w2"], dtype=np.float32)
            maps[c][f"emb_{t}_b2"] = np.asarray(
                inputs[f"{t}_b2"], dtype=np.float32).reshape(1, D)
    for d in DIRS:
        s = scheds[d]
        for c in range(NCORES):
            maps[c][f"sidx_{d}"] = s.src_idx[c]
            maps[c][f"didx_{d}"] = s.dst_idx[c]
            maps[c][f"widx_{d}"] = s.wrel_idx[c].astype(np.float32).astype(BF16)
            maps[c][f"efT_{d}"] = s.efp[c]
    iot = np.tile(np.arange(WIN, dtype=np.float32)[None, :], (128, 1))
    for c in range(NCORES):
        maps[c]["iota_row"] = iot.astype(BF16)
    for et, pre in (("vc", "ewvc"), ("vk", "ewvk")):
        w1 = np.asarray(inputs[f"{pre}_w1"], dtype=np.float32)
        b1 = np.asarray(inputs[f"{pre}_b1"], dtype=np.float32)
        w2 = np.asarray(inputs[f"{pre}_w2"], dtype=np.float32)
        b2 = np.asarray(inputs[f"{pre}_b2"], dtype=np.float32)
        w1e = np.concatenate([w1[2 * D:2 * D + EF], b1[None, :]], axis=0)
        for c in range(NCORES):
            maps[c][f"ew_{et}_w1v"] = np.ascontiguousarray(w1[0:D])
            maps[c][f"ew_{et}_w1o"] = np.ascontiguousarray(w1[D:2 * D])
            maps[c][f"ew_{et}_w1e"] = np.ascontiguousarray(w1e)
            maps[c][f"ew_{et}_w2t"] = np.tile(w2.reshape(1, D), (1, GCALL)).astype(
                np.float32)
            maps[c][f"ew_{et}_b2"] = b2.reshape(1, 1)
    for nm in ("mp_msg_w1", "mp_msg_w2"):
        a = np.asarray(inputs[nm], dtype=np.float32)
        for c in range(NCORES):
            maps[c][nm] = a
    for nm in ("mp_gate_w", "mp_upd_w1", "mp_upd_w2"):
        a = np.asarray(inputs[nm], dtype=np.float32).astype(BF16)
        for c in range(NCORES):
            maps[c][nm] = a
    for nm in ("mp_msg_b1", "mp_msg_b2", "mp_gate_b", "mp_upd_b1", "mp_upd_b2",
               "mp_ln_g", "mp_ln_b"):
        a = np.asarray(inputs[nm], dtype=np.float32).reshape(16, 1, D)
        if nm == "mp_ln_g":
            a = a * np.sqrt(float(D))
        for c in range(NCORES):
            maps[c][nm] = a
    return maps


_CACHE = {}


def kernel(**inputs):
    inputs = {k: np.asarray(v) for k, v in inputs.items()}
    scheds = build_schedules(inputs)
    key = "k"
    if key not in _CACHE:
        _CACHE[key] = build_kernel(scheds)
    nc = _CACHE[key]
    maps = make_inputs(inputs, scheds)
    res = run_bass_kernel_spmd(nc, maps, core_ids=list(range(NCORES)))
    out = np.concatenate([res.results[c]["out_cut"] for c in range(NCORES)], axis=0)
    return out.astype(np.float32)



# revision 32
# speedup vs baseline: 1.6456x; 1.6456x over previous
"""Trainium-2 Bass kernel for nn_EnhancedGNNEncoder (4-layer bipartite GNN).

8 NeuronCores, one SPMD Bass program. Nodes canonically sharded; edges
sharded by destination owner with a per-core-uniform chunk schedule
(64-node dst windows x 25k-row src banks). Message rows fetched with
dma_gather (4 SWDGE queues); segment-sum via host-built one-hot matrices
on the tensor engine accumulating [64,65] PSUM windows (degree in col 64);
per-pass message tables exchanged with AllGather collectives.
"""
import numpy as np
import ml_dtypes

import concourse.bacc as bacc
import concourse.mybir as mybir
from concourse.tile import TileContext
from concourse.bass_utils import run_bass_kernel_spmd
from concourse._compat import get_trn_type, cdiv
from concourse.library_config import mlp as mlp_lib
from concourse.masks import make_identity

BF16 = ml_dtypes.bfloat16
NCORES = 8
WIN = 64
BANK = 25000
GCALL = 4
D = 64
LN_EPS = 1e-3
LAYERS = 4
NV, NC_, NK = 100000, 50000, 20000
EF = 8
AF = mybir.ActivationFunctionType
ALU = mybir.AluOpType
AX = mybir.AxisListType

DIRS = ["vc0", "vc1", "vk0", "vk1"]
SRCT = {"vc0": "var", "vc1": "cons", "vk0": "var", "vk1": "cut"}
DSTT = {"vc0": "cons", "vc1": "var", "vk0": "cut", "vk1": "var"}
NTYPE = {"var": NV, "cons": NC_, "cut": NK}
FD = {"var": 19, "cons": 5, "cut": 30}
TSRC = {"vc0": "tvar_vc", "vc1": "tcons_vc", "vk0": "tvar_vk", "vk1": "tcut_vk"}
TDST = {"vc0": "tcons_vc", "vc1": "tvar_vc", "vk0": "tcut_vk", "vk1": "tvar_vk"}


def _wrap_idx(flat):
    n = len(flat)
    a = np.zeros((16, cdiv(n, 16)), dtype=np.int16)
    a[np.arange(n) % 16, np.arange(n) // 16] = flat
    return np.tile(a, (8, 1))


class DirSchedule:
    """Uniform schedule for one direction. Chunk streams are stored in
    *bank-stream* column order: col(k) = bank_base[bank(k)] + pos_in_bank(k)."""

    def __init__(self, src, dst, ef, n_src, n_dst, name):
        self.name, self.n_src, self.n_dst = name, n_src, n_dst
        self.shard = n_dst // NCORES
        self.nwin = cdiv(self.shard, WIN)
        self.nbank = cdiv(n_src, BANK)
        owner = dst // self.shard
        dloc = dst - owner * self.shard
        w_of = dloc // WIN
        b_of = src // BANK

        # (window, bank) cell sizes: 16-slot granular, max over cores,
        # floored at 128 so a 128-slot chunk never spans >2 windows.
        cnt = np.zeros((NCORES, self.nbank, self.nwin), dtype=np.int64)
        per = {}
        for c in range(NCORES):
            m = np.flatnonzero(owner == c)
            key = (b_of[m].astype(np.int64) * self.nwin + w_of[m]) * (
                2**18) + src[m]
            e = m[np.argsort(key, kind="stable")]
            per[c] = e
            np.add.at(cnt[c], (b_of[e], w_of[e]), 1)
        n_wb = np.maximum(128, 16 * ((cnt.max(axis=0) + 15) // 16))  # [nbank, nwin]
        off_wb = np.zeros_like(n_wb)
        self.bank_nchunk = []
        for bk in range(self.nbank):
            off_wb[bk] = np.concatenate([[0], np.cumsum(n_wb[bk])[:-1]])
            self.bank_nchunk.append(cdiv(int(n_wb[bk].sum()), 128))
        self.bank_base = np.concatenate([[0], np.cumsum(self.bank_nchunk)])[:-1]
        self.nchunk = int(sum(self.bank_nchunk))
        self.cpb = [int(x) for x in self.bank_nchunk]
        self.cpw = self.nchunk // max(1, self.nwin)

        # per-window matmul chunk lists (chunks the window's cells overlap)
        self.win_mms = []
        for wv in range(self.nwin):
            cols = []
            for bk in range(self.nbank):
                o, n = int(off_wb[bk, wv]), int(n_wb[bk, wv])
                c0 = int(self.bank_base[bk]) + o // 128
                c1 = int(self.bank_base[bk]) + (o + n - 1) // 128
                cols += list(range(c0, c1 + 1))
            self.win_mms.append(cols)

        self.efp = np.zeros((NCORES, EF + 1, self.nchunk * 128), dtype=np.float32)
        self.efp[:, EF, :] = 1.0
        self.src_idx, self.dst_idx, self.wrel_idx = [], [], []
        for c in range(NCORES):
            e = per[c]
            sflat = np.zeros(self.nchunk * 128, dtype=np.int16)
            dflat = np.zeros(self.nchunk * 128, dtype=np.int16)
            # pair-relative dloc (dloc - (w//2)*128 in [0,128)); pad slots -1
            wflat = np.full(self.nchunk * 128, -1, dtype=np.int16)
            ei = 0
            for bk in range(self.nbank):
                sbase = int(self.bank_base[bk]) * 128
                eb = e[b_of[e] == bk]
                ei = 0
                for wv in range(self.nwin):
                    sel = eb[ei:ei + int(cnt[c, bk, wv])]
                    ei += int(cnt[c, bk, wv])
                    n = len(sel)
                    if n:
                        p0 = sbase + int(off_wb[bk, wv])
                        self.efp[c, :EF, p0:p0 + n] = ef[sel].T
                        sflat[p0:p0 + n] = (src[sel] - bk * BANK).astype(np.int16)
                        dflat[p0:p0 + n] = dloc[sel].astype(np.int16)
                        wflat[p0:p0 + n] = (
                            dloc[sel] - (wv // 2) * 2 * WIN).astype(np.int16)
            self.src_idx.append(_wrap_idx(sflat))
            self.dst_idx.append(_wrap_idx(dflat))
            self.wrel_idx.append(wflat.reshape(self.nchunk, 128).T.copy())
        self.idx_cols = cdiv(self.nchunk * 128, 16)
        self.npiece = cdiv(self.nchunk, GCALL)


def build_schedules(inputs):
    vc, vk = inputs["var_cons_edges"], inputs["var_cut_edges"]
    efvc, efvk = inputs["var_cons_edge_features"], inputs["var_cut_edge_features"]
    return {
        "vc0": DirSchedule(vc[0], vc[1], efvc, NV, NC_, "vc0"),
        "vc1": DirSchedule(vc[1], vc[0], efvc, NC_, NV, "vc1"),
        "vk0": DirSchedule(vk[0], vk[1], efvk, NV, NK, "vk0"),
        "vk1": DirSchedule(vk[1], vk[0], efvk, NK, NV, "vk1"),
    }


def build_kernel(scheds):
    f32, bf16, i16 = mybir.dt.float32, mybir.dt.bfloat16, mybir.dt.int16
    nc = bacc.Bacc(get_trn_type() or "TRN2", num_swdge_queues=4)
    RG = [list(range(NCORES))]
    ins = {}

    def I(name, shape, dt=f32):
        ins[name] = nc.dram_tensor(name, shape, dt, kind="ExternalInput")
        return ins[name]

    for t in ("var", "cons", "cut"):
        I(f"featT_{t}", [FD[t] + 1, NTYPE[t] // NCORES])
        I(f"emb_{t}_w1", [FD[t] + 1, D])
        I(f"emb_{t}_w2", [D, D])
        I(f"emb_{t}_b2", [1, D])
    for d in DIRS:
        s = scheds[d]
        I(f"sidx_{d}", [128, s.idx_cols], i16)
        I(f"didx_{d}", [128, s.idx_cols], i16)
        I(f"widx_{d}", [128, s.nchunk], bf16)
        I(f"efT_{d}", [EF + 1, s.nchunk * 128])
    I("iota_row", [128, 2 * WIN], bf16)
    for et in ("vc", "vk"):
        I(f"ew_{et}_w1v", [D, D])
        I(f"ew_{et}_w1o", [D, D])
        I(f"ew_{et}_w1e", [EF + 1, D])
        I(f"ew_{et}_w2t", [1, GCALL * D])
        I(f"ew_{et}_b2", [1, 1])
    I("mp_msg_w1", [16, D, D]); I("mp_msg_b1", [16, 1, D])
    I("mp_msg_w2", [16, D, D]); I("mp_msg_b2", [16, 1, D])
    I("mp_gate_w", [16, 2 * D, D], bf16); I("mp_gate_b", [16, 1, D])
    I("mp_upd_w1", [16, 2 * D, D], bf16); I("mp_upd_b1", [16, 1, D])
    I("mp_upd_w2", [16, D, D], bf16); I("mp_upd_b2", [16, 1, D])
    I("mp_ln_g", [16, 1, D]); I("mp_ln_b", [16, 1, D])

    out_t = nc.dram_tensor("out_cut", [NK // NCORES, D], f32, kind="ExternalOutput")

    # hT node tables live in SBUF for the whole kernel (col offsets per type)
    HOFF = {"var": 0, "cons": NV // NCORES, "cut": NV // NCORES + NC_ // NCORES}
    HTOT = (NV + NC_ + NK) // NCORES
    tcc_in, tcc_out = {}, {}
    for nm, t in (("tvar_vc", "var"), ("tcons_vc", "cons"),
                  ("tvar_vk", "var"), ("tcut_vk", "cut")):
        tcc_in[nm] = nc.dram_tensor(f"cci_{nm}", [NTYPE[t] // NCORES, D], f32)
        tcc_out[nm] = nc.dram_tensor(f"cco_{nm}", [NTYPE[t], D], f32,
                                     addr_space="Shared")
    msg_in, msg_out = {}, {}
    for l in range(4):
        for d in DIRS:
            msg_in[(l, d)] = nc.dram_tensor(
                f"mci{l}{d}", [NTYPE[SRCT[d]] // NCORES, D], f32)
            msg_out[(l, d)] = nc.dram_tensor(
                f"mco{l}{d}", [NTYPE[SRCT[d]], D], f32, addr_space="Shared")

    with TileContext(nc) as tc:
        with (
            tc.tile_pool(name="const", bufs=1) as cp,
            tc.tile_pool(name="sb", bufs=2) as sb,
            tc.tile_pool(name="gp", bufs=2) as gp,
            tc.tile_pool(name="mgp", bufs=9) as mgp,
            tc.tile_pool(name="wp", bufs=9) as wp,
            tc.tile_pool(name="ps", bufs=2, space="PSUM") as ps,
            tc.tile_pool(name="psw", bufs=3, space="PSUM") as psw,
            tc.tile_pool(name="pst", bufs=2, space="PSUM") as pst,
        ):
            nc.gpsimd.load_library(mlp_lib)
            ident = cp.tile([128, 128], f32)
            make_identity(nc, ident)
            ones1 = cp.tile([1, 128], f32)
            nc.gpsimd.memset(ones1[:], 1.0)
            eps64 = cp.tile([128, 1], f32)
            nc.gpsimd.memset(eps64[:], float(D) * LN_EPS)
            inv64 = cp.tile([128, 1], f32)
            nc.gpsimd.memset(inv64[:], 1.0 / D)
            hT_sb = cp.tile([D, HTOT], f32)

            def hsl(t, a, b):
                return hT_sb[:, HOFF[t] + a:HOFF[t] + b]

            # iota row 0..2*WIN-1 on all partitions + per-direction
            # pair-relative dst indices (-1..127); small ints, exact in bf16
            iotaf = cp.tile([128, 2 * WIN], bf16)
            nc.sync.dma_start(iotaf[:], ins["iota_row"][:])
            didxw = {}
            for d in DIRS:
                s = scheds[d]
                dwt = cp.tile([128, s.nchunk], bf16, tag=f"dw{d}")
                nc.sync.dma_start(dwt[:], ins[f"widx_{d}"][:])
                didxw[d] = dwt

            def mmul(o, lt, r, st=True, sp=True):
                nc.tensor.matmul(o, lt, r, start=st, stop=sp)

            def col_of(row_dram, n, tag="colc"):
                rr = sb.tile([1, 128], f32, tag="colr")
                nc.sync.dma_start(rr[:, :n], row_dram)
                p = pst.tile([128, 128], f32, tag="tr")
                nc.tensor.transpose(p[:n, :1], rr[:1, :n], ident[:1, :1])
                c = sb.tile([128, 1], f32, tag=tag)
                nc.scalar.copy(c[:n, :], p[:n, :1])
                return c

            def bcast128(row_dram, n, tagsuf):
                rr = sb.tile([1, 128], f32, tag="bcr")
                nc.sync.dma_start(rr[:, :n], row_dram)
                p = pst.tile([128, 128], f32, tag="tr")
                mmul(p[:, :n], ones1[:], rr[:, :n])
                t = cp.tile([128, D], f32, tag="bc" + tagsuf)
                nc.vector.tensor_copy(t[:, :n], p[:, :n])
                return t

            # ---------------- P1: embeddings ----------------
            for t in ("var", "cons", "cut"):
                w1 = sb.tile([FD[t] + 1, D], f32, tag="ew1")
                nc.sync.dma_start(w1[:], ins[f"emb_{t}_w1"][:])
                w2 = sb.tile([D, D], f32, tag="ew2")
                nc.sync.dma_start(w2[:], ins[f"emb_{t}_w2"][:])
                b2c = col_of(ins[f"emb_{t}_b2"][:, :], D)
                sh = NTYPE[t] // NCORES
                for g0 in range(0, sh, 512):
                    gw = min(512, sh - g0)
                    ft = sb.tile([FD[t] + 1, 512], f32, tag="ft")
                    nc.sync.dma_start(ft[:, :gw], ins[f"featT_{t}"][:, g0:g0 + gw])
                    p1 = ps.tile([128, 512], f32, tag="mm")
                    mmul(p1[:D, :gw], w1[:], ft[:, :gw])
                    r1 = sb.tile([D, 512], f32, tag="r1")
                    nc.scalar.activation(r1[:, :gw], p1[:D, :gw], AF.Relu)
                    p2 = ps.tile([128, 512], f32, tag="mm")
                    mmul(p2[:D, :gw], w2[:], r1[:, :gw])
                    nc.vector.tensor_scalar_add(hsl(t, g0, g0 + gw),
                                                p2[:D, :gw], b2c[:D, :])

            # ---------------- P2a: t tables + AllGather ----------------
            for nm, t, wn in (("tvar_vc", "var", "ew_vc_w1v"),
                              ("tcons_vc", "cons", "ew_vc_w1o"),
                              ("tvar_vk", "var", "ew_vk_w1v"),
                              ("tcut_vk", "cut", "ew_vk_w1o")):
                wt = sb.tile([D, D], f32, tag="ew2")
                nc.sync.dma_start(wt[:], ins[wn][:])
                sh = NTYPE[t] // NCORES
                for g0 in range(0, sh, 512):
                    gw = min(512, sh - g0)
                    pt = ps.tile([128, 512], f32, tag="mm")
                    mmul(pt[:D, :gw], wt[:], hsl(t, g0, g0 + gw))
                    tt = sb.tile([D, 512], f32, tag="r1")
                    nc.vector.tensor_copy(tt[:, :gw], pt[:D, :gw])
                    for j0 in range(0, gw, 128):
                        jw = min(128, gw - j0)
                        pn = pst.tile([128, 128], f32, tag="tr")
                        nc.tensor.transpose(pn[:jw, :D], tt[:, j0:j0 + jw], ident[:D, :D])
                        nb = sb.tile([128, D], f32, tag="nb")
                        nc.scalar.copy(nb[:jw, :], pn[:jw, :D])
                        nc.sync.dma_start(tcc_in[nm][g0 + j0:g0 + j0 + jw, :],
                                          nb[:jw, :])
                nc.gpsimd.collective_compute(
                    "AllGather", ALU.bypass, ins=[tcc_in[nm][:]],
                    outs=[tcc_out[nm][:]], replica_groups=RG)

            # ---------------- P2b: edge weights ----------------

            ew_tiles, r_tiles = {}, {}
            for d in DIRS:
                s = scheds[d]
                et = "vc" if d[:2] == "vc" else "vk"
                w1e = sb.tile([EF + 1, D], f32, tag="w1e")
                nc.sync.dma_start(w1e[:], ins[f"ew_{et}_w1e"][:])
                w2b = cp.tile([128, GCALL, D], f32, tag=f"w2b{et}")
                w2r = sb.tile([1, GCALL * D], f32, tag="w2r")
                nc.sync.dma_start(w2r[:], ins[f"ew_{et}_w2t"][:])
                for hh in range(1):
                    pw = ps.tile([128, 512], f32, tag="mm")
                    mmul(pw[:, :GCALL * D], ones1[:], w2r[:])
                    nc.vector.tensor_copy(
                        w2b[:].rearrange("p a b -> p (a b)")[:],
                        pw[:, :GCALL * D])
                b2b = cp.tile([128, 1], f32, tag=f"eb2{et}")
                pb2 = ps.tile([128, 512], f32, tag="mm")
                b2r = sb.tile([1, 1], f32, tag="b2r1")
                nc.sync.dma_start(b2r[:], ins[f"ew_{et}_b2"][:])
                mmul(pb2[:, :1], ones1[:], b2r[:])
                nc.vector.tensor_copy(b2b[:], pb2[:, :1])

                ew_sb = cp.tile([128, s.nchunk], bf16, tag=f"ews{d}")
                ew_tiles[d] = ew_sb
                for p0 in range(0, s.nchunk, GCALL):
                    pn = min(GCALL, s.nchunk - p0)
                    # source-side gather: split by bank within the piece
                    sit = sb.tile([128, GCALL * 8], i16, tag="sit")
                    nc.sync.dma_start(sit[:, :pn * 8],
                                      ins[f"sidx_{d}"][:, p0 * 8:(p0 + pn) * 8])
                    dit = sb.tile([128, GCALL * 8], i16, tag="dit")
                    nc.sync.dma_start(dit[:, :pn * 8],
                                      ins[f"didx_{d}"][:, p0 * 8:(p0 + pn) * 8])
                    g1 = gp.tile([128, GCALL, D], f32, tag="sg")
                    c0 = p0
                    while c0 < p0 + pn:
                        bk = int(np.searchsorted(s.bank_base, c0, side="right") - 1)
                        bend = (s.bank_base[bk + 1] if bk + 1 < s.nbank
                                else s.nchunk)
                        cn = min(p0 + pn, bend) - c0
                        nrow = min(BANK, s.n_src - bk * BANK)
                        nc.gpsimd.dma_gather(
                            g1[:, c0 - p0:c0 - p0 + cn, :],
                            tcc_out[TSRC[d]][bk * BANK:bk * BANK + nrow, :],
                            sit[:, (c0 - p0) * 8:(c0 - p0 + cn) * 8],
                            cn * 128, cn * 128, D, single_packet=True,
                            queue_num=(c0 // GCALL) % 2)
                        c0 += cn
                    g2 = gp.tile([128, GCALL, D], f32, tag="dg")
                    nc.gpsimd.dma_gather(
                        g2[:, :pn, :], tcc_in[TDST[d]][:, :],
                        dit[:, :pn * 8],
                        pn * 128, pn * 128, D, single_packet=True,
                        queue_num=2 + (p0 // GCALL) % 2)
                    eft = sb.tile([EF + 1, GCALL * 128], f32, tag="eft")
                    nc.sync.dma_start(eft[:, :pn * 128],
                                      ins[f"efT_{d}"][:, p0 * 128:(p0 + pn) * 128])
                    pre = sb.tile([128, GCALL, D], f32, tag="pre")
                    for hh in range(0, pn, 8):
                        hn = min(8, pn - hh)
                        pe = ps.tile([128, 512], f32, tag="mm")
                        pev = pe[:].rearrange("p (a b) -> p a b", b=D)
                        for j in range(hn):
                            col = hh + j
                            mmul(pev[:, j, :],
                                 eft[:, col * 128:(col + 1) * 128],
                                 w1e[:])
                        nc.vector.tensor_tensor(
                            out=pre[:, hh:hh + hn, :], in0=g1[:, hh:hh + hn, :],
                            in1=pev[:, :hn, :], op=ALU.add)
                    nc.vector.tensor_tensor(out=pre[:, :pn, :], in0=pre[:, :pn, :],
                                            in1=g2[:, :pn, :], op=ALU.add)
                    rl = sb.tile([128, GCALL, D], f32, tag="rl")
                    nc.scalar.activation(rl[:, :pn, :], pre[:, :pn, :], AF.Relu)
                    nc.vector.tensor_tensor(out=rl[:, :pn, :], in0=rl[:, :pn, :],
                                            in1=w2b[:, :pn, :], op=ALU.mult)
                    sm = sb.tile([128, GCALL], f32, tag="sm")
                    nc.vector.reduce_sum(sm[:, :pn], rl[:, :pn, :], axis=AX.X)
                    nc.scalar.activation(ew_sb[:, p0:p0 + pn], sm[:, :pn],
                                         AF.Sigmoid, bias=b2b[:])
                rt_ = cp.tile([WIN, s.nwin], f32, tag=f"r{d}")
                r_tiles[d] = rt_

            # ---------------- P3: layers ----------------
            for l in range(LAYERS):
                for di, d in enumerate(DIRS):
                    i = l * 4 + di
                    s = scheds[d]
                    st, dt_ = SRCT[d], DSTT[d]
                    ssh, dsh = NTYPE[st] // NCORES, NTYPE[dt_] // NCORES

                    # --- msg mlp on own src shard, to node-major msg_in ---
                    mw1 = sb.tile([D, D], f32, tag="mw1")
                    nc.sync.dma_start(mw1[:], ins["mp_msg_w1"][i])
                    mw2 = sb.tile([D, D], f32, tag="mw2")
                    nc.sync.dma_start(mw2[:], ins["mp_msg_w2"][i])
                    mb1 = col_of(ins["mp_msg_b1"][i], D, "cmb1")
                    mb2 = col_of(ins["mp_msg_b2"][i], D, "cmb2")
                    for g0 in range(0, ssh, 512):
                        gw = min(512, ssh - g0)
                        p1 = ps.tile([128, 512], f32, tag="mm")
                        mmul(p1[:D, :gw], mw1[:], hsl(st, g0, g0 + gw))
                        r1 = sb.tile([D, 512], f32, tag="r1")
                        nc.scalar.activation(r1[:, :gw], p1[:D, :gw], AF.Relu,
                                             bias=mb1[:D, :])
                        p2 = ps.tile([128, 512], f32, tag="mm")
                        mmul(p2[:D, :gw], mw2[:], r1[:, :gw])
                        mt = sb.tile([D, 512], f32, tag="h1")
                        nc.vector.tensor_scalar_add(mt[:, :gw], p2[:D, :gw], mb2[:D, :])
                        for j0 in range(0, gw, 128):
                            jw = min(128, gw - j0)
                            pn_ = pst.tile([128, 128], f32, tag="tr")
                            nc.tensor.transpose(pn_[:jw, :D], mt[:, j0:j0 + jw],
                                                ident[:D, :D])
                            nb = sb.tile([128, D], f32, tag="nb")
                            nc.scalar.copy(nb[:jw, :], pn_[:jw, :D])
                            nc.sync.dma_start(
                                msg_in[(l, d)][g0 + j0:g0 + j0 + jw, :], nb[:jw, :])
                    nc.gpsimd.collective_compute(
                        "AllGather", ALU.bypass, ins=[msg_in[(l, d)][:]],
                        outs=[msg_out[(l, d)][:]], replica_groups=RG)

                    # --- update-phase constants (bf16 weights) ---
                    gwt = sb.tile([2 * D, D], bf16, tag="gwt")
                    nc.sync.dma_start(gwt[:], ins["mp_gate_w"][i])
                    uw1 = sb.tile([2 * D, D], bf16, tag="uw1")
                    nc.sync.dma_start(uw1[:], ins["mp_upd_w1"][i])
                    uw2 = sb.tile([D, D], bf16, tag="uw2")
                    nc.sync.dma_start(uw2[:], ins["mp_upd_w2"][i])
                    gb = col_of(ins["mp_gate_b"][i], D, "cgb")
                    ub1 = col_of(ins["mp_upd_b1"][i], D, "cub1")
                    ub2 = col_of(ins["mp_upd_b2"][i], D, "cub2")
                    lng = bcast128(ins["mp_ln_g"][i], D, "g")
                    lnb = bcast128(ins["mp_ln_b"][i], D, "b")

                    # --- edge phase + fused update every 8 windows ---
                    ew_sb, r_t = ew_tiles[d], r_tiles[d]
                    pieces = {}

                    use_ctr = [0]

                    def get_piece(pidx, d=d, s=s, ew_sb=ew_sb, pieces=pieces, l=l):
                        use_ctr[0] += 1
                        if pidx in pieces:
                            pieces[pidx][2] = use_ctr[0]
                            return pieces[pidx][:2]
                        p0 = pidx * GCALL
                        pn = min(GCALL, s.nchunk - p0)
                        sit = sb.tile([128, GCALL * 8], i16, tag="sit")
                        nc.sync.dma_start(sit[:, :pn * 8],
                                          ins[f"sidx_{d}"][:, p0 * 8:(p0 + pn) * 8])
                        g = mgp.tile([128, GCALL, D], f32, tag="mg")
                        c0 = p0
                        while c0 < p0 + pn:
                            bk = int(np.searchsorted(s.bank_base, c0, "right") - 1)
                            bend = (s.bank_base[bk + 1] if bk + 1 < s.nbank
                                    else s.nchunk)
                            cn = min(p0 + pn, bend) - c0
                            nrow = min(BANK, s.n_src - bk * BANK)
                            nc.gpsimd.dma_gather(
                                g[:, c0 - p0:c0 - p0 + cn, :],
                                msg_out[(l, d)][bk * BANK:bk * BANK + nrow, :],
                                sit[:, (c0 - p0) * 8:(c0 - p0 + cn) * 8],
                                cn * 128, cn * 128, D, single_packet=True,
                                queue_num=pidx % 4)
                            c0 += cn
                        wt = wp.tile([128, GCALL, 2 * WIN], bf16, tag="wt")
                        nc.vector.tensor_tensor(
                            out=wt[:, :pn, :],
                            in0=didxw[d][:, p0:p0 + pn, None].to_broadcast(
                                [128, pn, 2 * WIN]),
                            in1=iotaf[:, None, :].to_broadcast(
                                [128, pn, 2 * WIN]),
                            op=ALU.is_equal)
                        wm = wp.tile([128, GCALL, D + 1], bf16, tag="wm")
                        nc.vector.tensor_tensor(
                            out=wm[:, :pn, 0:D], in0=g[:, :pn, :],
                            in1=ew_sb[:, p0:p0 + pn, None].to_broadcast(
                                [128, pn, D]),
                            op=ALU.mult)
                        nc.vector.tensor_copy(wm[:, :pn, D], ew_sb[:, p0:p0 + pn])
                        pieces[pidx] = [wt, wm, use_ctr[0]]
                        if len(pieces) > 8:
                            lru = min((p for p in pieces if p != pidx),
                                      key=lambda p: pieces[p][2])
                            del pieces[lru]
                        return pieces[pidx][:2]

                    grp_aggT = None
                    for wv in range(s.nwin):
                        gi = wv % 8
                        if gi == 0:
                            grp_aggT = sb.tile([D, 512], f32, tag="gaggT")
                        pw = psw.tile([WIN, D + 1], f32, tag="win")
                        half = (wv % 2) * WIN
                        mms = s.win_mms[wv]
                        for j, col in enumerate(mms):
                            wt, wm = get_piece(col // GCALL)
                            sl = col % GCALL
                            mmul(pw[:], wt[:, sl, half:half + WIN],
                                 wm[:, sl, :],
                                 st=(j == 0), sp=(j == len(mms) - 1))
                        if l == 0:
                            dg = sb.tile([WIN, 1], f32, tag="dg1")
                            nc.vector.tensor_scalar_max(dg[:], pw[:, D:D + 1], 1.0)
                            nc.vector.reciprocal(r_t[:, wv:wv + 1], dg[:])
                        agg = sb.tile([WIN, D], f32, tag="aggnm")
                        nc.scalar.activation(agg[:], pw[:, 0:D], AF.Copy,
                                             scale=r_t[:, wv:wv + 1])
                        pt_ = pst.tile([128, 128], f32, tag="tr")
                        nc.tensor.transpose(pt_[:D, :WIN], agg[:], ident[:WIN, :WIN])
                        nc.vector.tensor_copy(grp_aggT[:, gi * WIN:(gi + 1) * WIN],
                                              pt_[:D, :WIN])

                        if gi == 7 or wv == s.nwin - 1:
                            ng = gi + 1
                            n0 = (wv - gi) * WIN
                            nn = ng * WIN
                            hTg = sb.tile([D, 512], f32, tag="hTg")
                            nc.vector.tensor_copy(
                                hTg[:, :min(nn, dsh - n0)],
                                hsl(dt_, n0, min(n0 + nn, dsh)))
                            combT = sb.tile([2 * D, 512], bf16, tag="combT")
                            nc.vector.tensor_copy(combT[0:D, :nn],
                                                  grp_aggT[:, :nn])
                            nc.vector.tensor_copy(combT[D:2 * D, :nn],
                                                  hTg[:, :nn])
                            pg = ps.tile([128, 512], f32, tag="mm")
                            mmul(pg[:D, :nn], gwt[:], combT[:, :nn])
                            gt = sb.tile([D, 512], f32, tag="gt")
                            nc.scalar.activation(gt[:, :nn], pg[:D, :nn],
                                                 AF.Sigmoid, bias=gb[:D, :])
                            pu = ps.tile([128, 512], f32, tag="mm")
                            mmul(pu[:D, :nn], uw1[:], combT[:, :nn])
                            ru = sb.tile([D, 512], bf16, tag="ru")
                            nc.scalar.activation(ru[:, :nn], pu[:D, :nn], AF.Relu,
                                                 bias=ub1[:D, :])
                            pu2 = ps.tile([128, 512], f32, tag="mm")
                            mmul(pu2[:D, :nn], uw2[:], ru[:, :nn])
                            ut = sb.tile([D, 512], f32, tag="ut")
                            nc.vector.tensor_scalar_add(ut[:, :nn], pu2[:D, :nn],
                                                        ub2[:D, :])
                            # out = h + g*(u - h)
                            nc.vector.tensor_tensor(out=ut[:, :nn], in0=ut[:, :nn],
                                                    in1=hTg[:, :nn],
                                                    op=ALU.subtract)
                            nc.vector.tensor_tensor(out=ut[:, :nn], in0=ut[:, :nn],
                                                    in1=gt[:, :nn], op=ALU.mult)
                            nc.vector.tensor_tensor(out=ut[:, :nn], in0=ut[:, :nn],
                                                    in1=hTg[:, :nn],
                                                    op=ALU.add)
                            # (residual uses the f32 hTg staging copy)
                            # LN: per-window node-major blocks on 64 partitions
                            xb = sb.tile([WIN, 8, D], f32, tag="xb")
                            for w2 in range(ng):
                                ptp = pst.tile([128, 128], f32, tag="tr")
                                nc.tensor.transpose(
                                    ptp[0:WIN, :D],
                                    ut[:, w2 * WIN:(w2 + 1) * WIN], ident[:D, :D])
                                nc.vector.tensor_copy(xb[:, w2, :], ptp[:WIN, :D])
                            m_ = sb.tile([WIN, 8], f32, tag="mln")
                            nc.vector.reduce_sum(m_[:, :ng], xb[:, :ng, :],
                                                 axis=AX.X)
                            nc.vector.tensor_scalar_mul(m_[:, :ng],
                                                        m_[:, :ng], inv64[:WIN, :])
                            for w2 in range(ng):
                                nc.vector.tensor_scalar_sub(xb[:, w2, :],
                                                            xb[:, w2, :],
                                                            m_[:, w2:w2 + 1])
                            sq = sb.tile([WIN, 8, D], f32, tag="sq")
                            nc.scalar.activation(sq[:, :ng, :], xb[:, :ng, :],
                                                 AF.Square)
                            v_ = sb.tile([WIN, 8], f32, tag="vln")
                            nc.vector.reduce_sum(v_[:, :ng], sq[:, :ng, :],
                                                 axis=AX.X)
                            nc.vector.tensor_scalar_add(v_[:, :ng],
                                                        v_[:, :ng], eps64[:WIN, :])
                            nc.scalar.activation(v_[:, :ng], v_[:, :ng], AF.Sqrt)
                            nc.vector.reciprocal(v_[:, :ng], v_[:, :ng])
                            for w2 in range(ng):
                                nc.vector.tensor_scalar_mul(xb[:, w2, :],
                                                            xb[:, w2, :],
                                                            v_[:, w2:w2 + 1])
                            nc.vector.tensor_tensor(out=xb[:, :ng, :],
                                                    in0=xb[:, :ng, :],
                                                    in1=lng[:WIN, None, :].to_broadcast(
                                                        [WIN, ng, D]),
                                                    op=ALU.mult)
                            nc.vector.tensor_tensor(out=xb[:, :ng, :],
                                                    in0=xb[:, :ng, :],
                                                    in1=lnb[:WIN, None, :].to_broadcast(
                                                        [WIN, ng, D]),
                                                    op=ALU.add)
                            # transpose back to feature-major, store into hT_sb
                            for w2 in range(ng):
                                c0_ = n0 + w2 * WIN
                                cw_ = min(WIN, dsh - c0_)
                                if cw_ <= 0:
                                    break
                                pth = pst.tile([128, 128], f32, tag="tr")
                                nc.tensor.transpose(pth[:D, :WIN], xb[:, w2, :],
                                                    ident[:WIN, :WIN])
                                nc.vector.tensor_copy(
                                    hsl(dt_, c0_, c0_ + cw_), pth[:D, :cw_])

            # ---------------- output: h_cut node-major ----------------
            csh = NK // NCORES
            for g0 in range(0, csh, 512):
                gw = min(512, csh - g0)
                for j0 in range(0, gw, 128):
                    jw = min(128, gw - j0)
                    po = pst.tile([128, 128], f32, tag="tr")
                    nc.tensor.transpose(po[:jw, :D],
                                        hsl("cut", g0 + j0, g0 + j0 + jw),
                                        ident[:D, :D])
                    nb = sb.tile([128, D], f32, tag="nb")
                    nc.scalar.copy(nb[:jw, :], po[:jw, :D])
                    nc.sync.dma_start(out_t[g0 + j0:g0 + j0 + jw, :], nb[:jw, :])

    nc.compile()
    return nc


def make_inputs(inputs, scheds):
    """Build the per-core input maps from the model inputs + schedules."""
    maps = [dict() for _ in range(NCORES)]
    feats = {"var": "variable_features", "cons": "constraint_features",
             "cut": "cut_features"}
    for t in ("var", "cons", "cut"):
        f = np.asarray(inputs[feats[t]], dtype=np.float32)
        sh = NTYPE[t] // NCORES
        w1 = np.concatenate([np.asarray(inputs[f"{t}_w1"]),
                             np.asarray(inputs[f"{t}_b1"])[None, :]], axis=0)
        for c in range(NCORES):
            ft = np.ones((FD[t] + 1, sh), dtype=np.float32)
            ft[:FD[t], :] = f[c * sh:(c + 1) * sh].T
            maps[c][f"featT_{t}"] = ft
            maps[c][f"emb_{t}_w1"] = np.ascontiguousarray(w1, dtype=np.float32)
            maps[c][f"emb_{t}_w2"] = np.asarray(inputs[f"{t}_w2"], dtype=np.float32)
            maps[c][f"emb_{t}_b2"] = np.asarray(
                inputs[f"{t}_b2"], dtype=np.float32).reshape(1, D)
    for d in DIRS:
        s = scheds[d]
        for c in range(NCORES):
            maps[c][f"sidx_{d}"] = s.src_idx[c]
            maps[c][f"didx_{d}"] = s.dst_idx[c]
            maps[c][f"widx_{d}"] = s.wrel_idx[c].astype(np.float32).astype(BF16)
            maps[c][f"efT_{d}"] = s.efp[c]
    iot = np.tile(np.arange(2 * WIN, dtype=np.float32)[None, :], (128, 1))
    for c in range(NCORES):
        maps[c]["iota_row"] = iot.astype(BF16)
    for et, pre in (("vc", "ewvc"), ("vk", "ewvk")):
        w1 = np.asarray(inputs[f"{pre}_w1"], dtype=np.float32)
        b1 = np.asarray(inputs[f"{pre}_b1"], dtype=np.float32)
        w2 = np.asarray(inputs[f"{pre}_w2"], dtype=np.float32)
        b2 = np.asarray(inputs[f"{pre}_b2"], dtype=np.float32)
        w1e = np.concatenate([w1[2 * D:2 * D + EF], b1[None, :]], axis=0)
        for c in range(NCORES):
            maps[c][f"ew_{et}_w1v"] = np.ascontiguousarray(w1[0:D])
            maps[c][f"ew_{et}_w1o"] = np.ascontiguousarray(w1[D:2 * D])
            maps[c][f"ew_{et}_w1e"] = np.ascontiguousarray(w1e)
            maps[c][f"ew_{et}_w2t"] = np.tile(w2.reshape(1, D), (1, GCALL)).astype(
                np.float32)
            maps[c][f"ew_{et}_b2"] = b2.reshape(1, 1)
    for nm in ("mp_msg_w1", "mp_msg_w2"):
        a = np.asarray(inputs[nm], dtype=np.float32)
        for c in range(NCORES):
            maps[c][nm] = a
    for nm in ("mp_gate_w", "mp_upd_w1", "mp_upd_w2"):
        a = np.asarray(inputs[nm], dtype=np.float32).astype(BF16)
        for c in range(NCORES):
            maps[c][nm] = a
    for nm in ("mp_msg_b1", "mp_msg_b2", "mp_gate_b", "mp_upd_b1", "mp_upd_b2",
               "mp_ln_g", "mp_ln_b"):
        a = np.asarray(inputs[nm], dtype=np.float32).reshape(16, 1, D)
        if nm == "mp_ln_g":
            a = a * np.sqrt(float(D))
        for c in range(NCORES):
            maps[c][nm] = a
    return maps


_CACHE = {}


def kernel(**inputs):
    inputs = {k: np.asarray(v) for k, v in inputs.items()}
    scheds = build_schedules(inputs)
    key = "k"
    if key not in _CACHE:
        _CACHE[key] = build_kernel(scheds)
    nc = _CACHE[key]
    maps = make_inputs(inputs, scheds)
    res = run_bass_kernel_spmd(nc, maps, core_ids=list(range(NCORES)))
    out = np.concatenate([res.results[c]["out_cut"] for c in range(NCORES)], axis=0)
    return out.astype(np.float32)

